# revision 7
# baseline (speedup 1.0000x reference)
"""Trainium2 Bass kernel for the patch-GP conditional (conv GP layer).

Contract: kernel(**inputs) takes the FULL inputs (as produced by
setup_inputs()) and returns the FULL output (mean, var), each [N, P*G].

Math (equivalent to the reference's whitened-free conditional):
    Kuf[g,m,x]  = variance * exp(-0.5*(||z_m||^2 + ||x_x||^2 - 2 z_m.x_x)/ls^2)
    fmean[g,x]  = sum_m d_g[m] * Kuf[g,m,x],          d_g   = Kuu_g^{-1} q_mu[:,g]
    fvar[g,x]   = variance - sum_k Kuf[g,k,x] * (Q_g @ Kuf[g])[k,x]
    Q_g         = Kuu_g^{-1} - (Kuu_g^{-1} Lq_g)(Kuu_g^{-1} Lq_g)^T
Host does the tiny O(M^3) prep in float64 (Kuu, its inverse, Q, d) plus
patch extraction / layout; the 8 NeuronCores each do the O(M * Ploc*N)
work for their shard of the patch dimension P.

Device layout per core (x = ploc*N + n, Xloc = 98*32 = 3136 columns):
    xt    [76, 3136]  rows 0..74 = patch vectors (L-major), row 75 = ||x||^2
    zaug  [128, G, 384] rows 0..74 = -2*Z[g,m,l], row 75 = 1, rows 76.. = 0
    qmat  [128, G, 3, 384]  Q_g tiles: [j, g, mo, k] = Q_g[mo*128+j, k]
    dv,bv [128, G*3]  d and exp-bias per m (col = g*3 + mo)
    ones  [128, 1]
    out   [2, G, 3136]  (mean/var, g, x)
"""

import numpy as np

# Problem constants (hardcoded per the task contract).
H = 32
W = 32
C = 3
PH = 5
PW = 5
JITTER = 1e-6
N = 32
G = 2
M = 384
L = PH * PW * C  # 75
P = (H - PH + 1) * (W - PW + 1)  # 784
NCORES = 8
PLOC = P // NCORES  # 98
XL = PLOC * N  # 3136
CHW = 448  # free-dim chunk width (PSUM bank holds 512 fp32)
NCH = XL // CHW  # 7
MT = M // 128  # 3 partition tiles of the inducing dim

_CACHE = {}


def _ensure_concourse():
    try:
        import concourse  # noqa: F401
    except ImportError:
        import sys

        for p in ("/opt/trn_rl_repo", "/root/.axon_site/_ro/trn_rl_repo"):
            if p not in sys.path:
                sys.path.insert(0, p)


def _build(scale_imm: float, var_imm: float):
    """Build + compile the single-core SPMD program (same NEFF on all cores)."""
    _ensure_concourse()
    from concourse import bacc, mybir, tile

    f32 = mybir.dt.float32
    f32r = mybir.dt.float32r
    EXP = mybir.ActivationFunctionType.Exp
    COPY = mybir.ActivationFunctionType.Copy

    nc = bacc.Bacc("TRN2", target_bir_lowering=False, debug=False)

    xt = nc.dram_tensor("xt", [76, XL], f32r, kind="ExternalInput").ap()
    zaug = nc.dram_tensor("zaug", [76, G, M], f32r, kind="ExternalInput").ap()
    qmat = nc.dram_tensor("qmat", [128, G, MT, M], f32r, kind="ExternalInput").ap()
    dv = nc.dram_tensor("dv", [128, G * MT], f32r, kind="ExternalInput").ap()
    bv = nc.dram_tensor("bv", [128, G * MT], f32, kind="ExternalInput").ap()
    ones = nc.dram_tensor("ones", [128, 1], f32r, kind="ExternalInput").ap()
    out = nc.dram_tensor("out", [2 * G, XL], f32, kind="ExternalOutput").ap()

    with tile.TileContext(nc) as tc:
        with (
            tc.tile_pool(name="const", bufs=1) as const,
            tc.tile_pool(name="work", bufs=2) as work,
            tc.tile_pool(name="ps", bufs=2, space="PSUM") as ps,
        ):
            xaug = const.tile([76, XL], f32r)
            nc.sync.dma_start(out=xaug, in_=xt)
            zsb = const.tile([76, G, M], f32r)
            nc.sync.dma_start(out=zsb, in_=zaug)
            qsb = const.tile([128, G, MT, M], f32r)
            nc.sync.dma_start(out=qsb, in_=qmat)
            dsb = const.tile([128, G * MT], f32r)
            nc.sync.dma_start(out=dsb, in_=dv)
            bsb = const.tile([128, G * MT], f32)
            nc.sync.dma_start(out=bsb, in_=bv)
            osb = const.tile([128, 1], f32r)
            nc.sync.dma_start(out=osb, in_=ones)

            macc = [const.tile([1, XL], f32, name=f"macc{g}") for g in range(G)]
            vacc = [const.tile([1, XL], f32, name=f"vacc{g}") for g in range(G)]

            for ch in range(NCH):
                sl = slice(ch * CHW, (ch + 1) * CHW)
                for g in range(G):
                    kufs = []
                    for mt in range(MT):
                        psq = ps.tile([128, CHW], f32, tag="psq", name="psq")
                        nc.tensor.matmul(
                            psq,
                            zsb[:, g, mt * 128 : (mt + 1) * 128],
                            xaug[:, sl],
                        )
                        kuf = work.tile([128, CHW], f32r, tag=f"kuf{mt}", name=f"kuf{mt}")
                        nc.scalar.activation(
                            kuf,
                            psq,
                            EXP,
                            bias=bsb[:, g * MT + mt : g * MT + mt + 1],
                            scale=scale_imm,
                        )
                        kufs.append(kuf)
                    pv = ps.tile([1, CHW], f32, tag="pv", name="pv")
                    pm = ps.tile([1, CHW], f32, tag="pm", name="pm")
                    for kt in range(MT):
                        pr = ps.tile([128, CHW], f32, tag="pr", name="pr")
                        for mt in range(MT):
                            nc.tensor.matmul(
                                pr,
                                qsb[:, g, mt, kt * 128 : (kt + 1) * 128],
                                kufs[mt],
                                start=(mt == 0),
                                stop=(mt == MT - 1),
                            )
                        pk = work.tile([128, CHW], f32r, tag="pk", name="pk")
                        nc.vector.tensor_mul(pk, kufs[kt], pr)
                        nc.tensor.matmul(
                            pv,
                            osb,
                            pk,
                            start=(kt == 0),
                            stop=(kt == MT - 1),
                        )
                        nc.tensor.matmul(
                            pm,
                            dsb[:, g * MT + kt : g * MT + kt + 1],
                            kufs[kt],
                            start=(kt == 0),
                            stop=(kt == MT - 1),
                        )
                    # fvar = variance - pv ; fmean = pm
                    nc.scalar.activation(
                        vacc[g][:, sl], pv, COPY, bias=var_imm, scale=-1.0
                    )
                    nc.vector.tensor_copy(macc[g][:, sl], pm)

            for g in range(G):
                nc.sync.dma_start(out=out[g : g + 1, :], in_=macc[g][0:1, :])
                nc.sync.dma_start(out=out[G + g : G + g + 1, :], in_=vacc[g][0:1, :])

    nc.compile()
    return nc


def _get_nc(scale_imm: float, var_imm: float):
    key = (round(scale_imm, 12), round(var_imm, 12))
    if key not in _CACHE:
        _CACHE[key] = _build(scale_imm, var_imm)
    return _CACHE[key]


def _host_prep(ND_X, Z, q_mu, q_sqrt, variance, lengthscale):
    from numpy.lib.stride_tricks import sliding_window_view

    ls = float(lengthscale)
    var = float(variance)
    scale = -0.5 / (ls * ls)

    x = np.asarray(ND_X, np.float32).reshape(N, H, W, C)
    swv = sliding_window_view(x, (PH, PW), axis=(1, 2))  # [N,28,28,C,5,5]
    pats = np.ascontiguousarray(swv.transpose(0, 1, 2, 4, 5, 3)).reshape(N, P, L)
    PNL = np.ascontiguousarray(pats.transpose(1, 0, 2))  # [P,N,L] float32

    Z64 = np.asarray(Z, np.float64)
    zsq = np.einsum("gml,gml->gm", Z64, Z64)  # [G,M]
    sqd = zsq[:, :, None] + zsq[:, None, :] - 2.0 * np.einsum(
        "gml,gnl->gmn", Z64, Z64
    )
    Kuu = var * np.exp(0.5 * sqd / (-ls * ls)) + JITTER * np.eye(M)
    Kinv = np.linalg.inv(Kuu)  # [G,M,M]
    Lq = np.tril(np.asarray(q_sqrt, np.float64))
    Bm = np.einsum("gmn,gnk->gmk", Kinv, Lq)
    Q = Kinv - np.einsum("gmk,gnk->gmn", Bm, Bm)  # [G,M,M]
    d = np.einsum("gmn,ng->gm", Kinv, np.asarray(q_mu, np.float64))  # [G,M]
    bias = scale * zsq + np.log(var)  # [G,M]

    zaug_h = np.zeros([76, G, M], np.float32)
    zaug_h[:L] = (-2.0 * Z64).transpose(2, 0, 1).astype(np.float32)
    zaug_h[L] = 1.0
    qmat_h = np.ascontiguousarray(
        Q.reshape(G, MT, 128, M).transpose(2, 0, 1, 3)
    ).astype(np.float32)
    dv_h = np.ascontiguousarray(
        d.reshape(G, MT, 128).transpose(2, 0, 1)
    ).reshape(128, G * MT).astype(np.float32)
    bv_h = np.ascontiguousarray(
        bias.reshape(G, MT, 128).transpose(2, 0, 1)
    ).reshape(128, G * MT).astype(np.float32)
    ones_h = np.ones([128, 1], np.float32)

    shared = {
        "zaug": zaug_h,
        "qmat": qmat_h,
        "dv": dv_h,
        "bv": bv_h,
        "ones": ones_h,
    }
    in_maps = []
    for c in range(NCORES):
        Xc = PNL[c * PLOC : (c + 1) * PLOC].reshape(XL, L)
        xt_h = np.empty([76, XL], np.float32)
        xt_h[:L] = Xc.T
        xt_h[L] = np.einsum("xl,xl->x", Xc.astype(np.float64), Xc.astype(np.float64))
        in_maps.append({"xt": np.ascontiguousarray(xt_h), **shared})
    return in_maps, scale, var


def _run(inputs, trace=False, trace_kwargs=None):
    _ensure_concourse()
    from concourse.bass_utils import run_bass_kernel_spmd

    in_maps, scale, var = _host_prep(**inputs)
    nc = _get_nc(scale, var)
    bkr = run_bass_kernel_spmd(
        nc,
        in_maps,
        list(range(NCORES)),
        trace=trace,
        **(trace_kwargs or {}),
    )
    mean = np.empty([N, P * G], np.float32)
    varr = np.empty([N, P * G], np.float32)
    for c in range(NCORES):
        o = np.asarray(bkr.results[c]["out"])  # [2*G, XL]: rows 0..G-1 mean, G..2G-1 var
        mean[:, c * PLOC * G : (c + 1) * PLOC * G] = (
            o[:G].reshape(G, PLOC, N).transpose(2, 1, 0).reshape(N, PLOC * G)
        )
        varr[:, c * PLOC * G : (c + 1) * PLOC * G] = (
            o[G:].reshape(G, PLOC, N).transpose(2, 1, 0).reshape(N, PLOC * G)
        )
    return mean, varr, bkr


def kernel(**inputs):
    mean, varr, _ = _run(inputs, trace=False)
    return mean, varr


# revision 8
# speedup vs baseline: 1.1218x; 1.1218x over previous
"""Trainium2 Bass kernel for the patch-GP conditional (conv GP layer).

Contract: kernel(**inputs) takes the FULL inputs (as produced by
setup_inputs()) and returns the FULL output (mean, var), each [N, P*G].

Math (equivalent to the reference's whitened-free conditional):
    Kuf[g,m,x]  = variance * exp(-0.5*(||z_m||^2 + ||x_x||^2 - 2 z_m.x_x)/ls^2)
                = cs[x] * kt[g,m,x],   cs[x] = exp(-0.5*||x_x||^2/ls^2)
    kt[g,m,x]   = exp(-0.5*(||z_m||^2 - 2 z_m.x_x)/ls^2 + ln(variance))
    fmean[g,x]  = cs[x] * sum_m d_g[m] kt[g,m,x],   d_g = Kuu_g^{-1} q_mu[:,g]
    fvar[g,x]   = variance - cs[x]^2 * sum_k kt[g,k,x] (Q_g @ kt[g])[k,x]
    Q_g         = Kuu_g^{-1} - (Kuu_g^{-1} Lq_g)(Kuu_g^{-1} Lq_g)^T
Host does the tiny O(M^3) prep in float64 (Kuu, inverse, Q, d), the patch
extraction / layout, and the final per-column cs/cs^2 rescale; the 8
NeuronCores each do the O(M * Ploc*N) work for their shard of P.

Device per core (x = ploc*N + n, Xloc = 98*32 = 3136 columns):
    sq   = zsb[:,g,mt]^T @ xaug          (3 matmuls / (g,chunk), K=75)
    kt   = exp(scale*sq + bias_m)        (ACT, per-partition bias)
    R    = Q @ kt                        (9 matmuls / (g,chunk))
    pacc = sum_kt (kt .* R)              (DVE mul+add)
    pv   = ones^T pacc                   (1 matmul)
    pm   = d^T kt                        (3 matmuls)
    out rows: [pm_g0, pm_g1, pv_g0, pv_g1]  (raw, host rescales)
"""

import numpy as np

# Problem constants (hardcoded per the task contract).
H = 32
W = 32
C = 3
PH = 5
PW = 5
JITTER = 1e-6
N = 32
G = 2
M = 384
L = PH * PW * C  # 75
P = (H - PH + 1) * (W - PW + 1)  # 784
NCORES = 8
PLOC = P // NCORES  # 98
XL = PLOC * N  # 3136
CHW = 448  # free-dim chunk width (PSUM bank holds 512 fp32)
NCH = XL // CHW  # 7
MT = M // 128  # 3 partition tiles of the inducing dim

# "bf16": TensorE at 1 cycle/row; "f32r": relaxed fp32, 2 cycles/row.
MODE = "bf16"

_CACHE = {}


def _ensure_concourse():
    try:
        import concourse  # noqa: F401
    except ImportError:
        import sys

        for p in ("/opt/trn_rl_repo", "/root/.axon_site/_ro/trn_rl_repo"):
            if p not in sys.path:
                sys.path.insert(0, p)


def _np_dt(mode):
    if mode == "bf16":
        import ml_dtypes

        return ml_dtypes.bfloat16
    return np.float32


def _build(scale_imm: float, mode: str):
    """Build + compile the single-core SPMD program (same NEFF on all cores)."""
    _ensure_concourse()
    from concourse import bacc, mybir, tile

    f32 = mybir.dt.float32
    DT = mybir.dt.bfloat16 if mode == "bf16" else mybir.dt.float32r
    EXP = mybir.ActivationFunctionType.Exp

    nc = bacc.Bacc("TRN2", target_bir_lowering=False, debug=False)

    xt = nc.dram_tensor("xt", [L, XL], DT, kind="ExternalInput").ap()
    zaug = nc.dram_tensor("zaug", [L, G, M], DT, kind="ExternalInput").ap()
    qmat = nc.dram_tensor("qmat", [128, G, MT, M], DT, kind="ExternalInput").ap()
    dv = nc.dram_tensor("dv", [128, G * MT], DT, kind="ExternalInput").ap()
    bv = nc.dram_tensor("bv", [128, G * MT], f32, kind="ExternalInput").ap()
    ones = nc.dram_tensor("ones", [128, 1], DT, kind="ExternalInput").ap()
    out = nc.dram_tensor("out", [2 * G, XL], f32, kind="ExternalOutput").ap()

    with tile.TileContext(nc) as tc:
        with (
            tc.tile_pool(name="const", bufs=1) as const,
            tc.tile_pool(name="work", bufs=2) as work,
            tc.tile_pool(name="ps", bufs=2, space="PSUM") as ps,
        ):
            xaug = const.tile([L, XL], DT)
            nc.sync.dma_start(out=xaug, in_=xt)
            zsb = const.tile([L, G, M], DT)
            nc.sync.dma_start(out=zsb, in_=zaug)
            qsb = const.tile([128, G, MT, M], DT)
            nc.sync.dma_start(out=qsb, in_=qmat)
            dsb = const.tile([128, G * MT], DT)
            nc.sync.dma_start(out=dsb, in_=dv)
            bsb = const.tile([128, G * MT], f32)
            nc.sync.dma_start(out=bsb, in_=bv)
            osb = const.tile([128, 1], DT)
            nc.sync.dma_start(out=osb, in_=ones)

            macc = [const.tile([1, XL], f32, name=f"macc{g}") for g in range(G)]
            vacc = [const.tile([1, XL], f32, name=f"vacc{g}") for g in range(G)]

            for ch in range(NCH):
                sl = slice(ch * CHW, (ch + 1) * CHW)
                for g in range(G):
                    kufs = []
                    for mt in range(MT):
                        psq = ps.tile([128, CHW], f32, tag="psq", name="psq")
                        nc.tensor.matmul(
                            psq,
                            zsb[:, g, mt * 128 : (mt + 1) * 128],
                            xaug[:, sl],
                        )
                        kuf = work.tile([128, CHW], DT, tag=f"kuf{mt}", name=f"kuf{mt}")
                        nc.scalar.activation(
                            kuf,
                            psq,
                            EXP,
                            bias=bsb[:, g * MT + mt : g * MT + mt + 1],
                            scale=scale_imm,
                        )
                        kufs.append(kuf)
                    pacc = work.tile([128, CHW], DT, tag="pacc", name="pacc")
                    for kt in range(MT):
                        pr = ps.tile([128, CHW], f32, tag="pr", name="pr")
                        for mt in range(MT):
                            nc.tensor.matmul(
                                pr,
                                qsb[:, g, mt, kt * 128 : (kt + 1) * 128],
                                kufs[mt],
                                start=(mt == 0),
                                stop=(mt == MT - 1),
                            )
                        if kt == 0:
                            nc.vector.tensor_mul(pacc, kufs[kt], pr)
                        else:
                            pk = work.tile([128, CHW], DT, tag="pk", name="pk")
                            nc.vector.tensor_mul(pk, kufs[kt], pr)
                            nc.vector.tensor_add(pacc, pacc, pk)
                    pvp = ps.tile([1, CHW], f32, tag="pvp", name="pvp")
                    nc.tensor.matmul(pvp, osb, pacc)
                    pmp = ps.tile([1, CHW], f32, tag="pmp", name="pmp")
                    for kt in range(MT):
                        nc.tensor.matmul(
                            pmp,
                            dsb[:, g * MT + kt : g * MT + kt + 1],
                            kufs[kt],
                            start=(kt == 0),
                            stop=(kt == MT - 1),
                        )
                    nc.vector.tensor_copy(vacc[g][:, sl], pvp)
                    nc.scalar.copy(macc[g][:, sl], pmp)

            for g in range(G):
                nc.sync.dma_start(out=out[g : g + 1, :], in_=macc[g][0:1, :])
                nc.sync.dma_start(out=out[G + g : G + g + 1, :], in_=vacc[g][0:1, :])

    nc.compile()
    return nc


def _get_nc(scale_imm: float, mode: str):
    key = (round(scale_imm, 12), mode)
    if key not in _CACHE:
        _CACHE[key] = _build(scale_imm, mode)
    return _CACHE[key]


def _host_prep(ND_X, Z, q_mu, q_sqrt, variance, lengthscale, mode):
    from numpy.lib.stride_tricks import sliding_window_view

    ls = float(lengthscale)
    var = float(variance)
    scale = -0.5 / (ls * ls)
    ndt = _np_dt(mode)

    x = np.asarray(ND_X, np.float32).reshape(N, H, W, C)
    swv = sliding_window_view(x, (PH, PW), axis=(1, 2))  # [N,28,28,C,5,5]
    pats = np.ascontiguousarray(swv.transpose(0, 1, 2, 4, 5, 3)).reshape(N, P, L)
    PNL = np.ascontiguousarray(pats.transpose(1, 0, 2))  # [P,N,L] float32

    Z64 = np.asarray(Z, np.float64)
    zsq = np.einsum("gml,gml->gm", Z64, Z64)  # [G,M]
    sqd = zsq[:, :, None] + zsq[:, None, :] - 2.0 * np.einsum(
        "gml,gnl->gmn", Z64, Z64
    )
    Kuu = var * np.exp(0.5 * sqd / (-ls * ls)) + JITTER * np.eye(M)
    Kinv = np.linalg.inv(Kuu)  # [G,M,M]
    Lq = np.tril(np.asarray(q_sqrt, np.float64))
    Bm = np.einsum("gmn,gnk->gmk", Kinv, Lq)
    Q = Kinv - np.einsum("gmk,gnk->gmn", Bm, Bm)  # [G,M,M]
    d = np.einsum("gmn,ng->gm", Kinv, np.asarray(q_mu, np.float64))  # [G,M]
    bias = scale * zsq + np.log(var)  # [G,M]

    zaug_h = np.ascontiguousarray(
        (-2.0 * Z64).transpose(2, 0, 1)
    ).astype(ndt)  # [L,G,M]
    qmat_h = np.ascontiguousarray(
        Q.reshape(G, MT, 128, M).transpose(2, 0, 1, 3)
    ).astype(ndt)
    dv_h = np.ascontiguousarray(
        d.reshape(G, MT, 128).transpose(2, 0, 1)
    ).reshape(128, G * MT).astype(ndt)
    bv_h = np.ascontiguousarray(
        bias.reshape(G, MT, 128).transpose(2, 0, 1)
    ).reshape(128, G * MT).astype(np.float32)
    ones_h = np.ones([128, 1], ndt)

    shared = {
        "zaug": zaug_h,
        "qmat": qmat_h,
        "dv": dv_h,
        "bv": bv_h,
        "ones": ones_h,
    }
    in_maps = []
    cs_all = []  # per-core per-column exp(scale*||x||^2), float64
    for c in range(NCORES):
        Xc = PNL[c * PLOC : (c + 1) * PLOC].reshape(XL, L)
        xt_h = np.ascontiguousarray(Xc.T).astype(ndt)
        xsq = np.einsum(
            "xl,xl->x", Xc.astype(np.float64), Xc.astype(np.float64)
        )
        cs_all.append(np.exp(scale * xsq))
        in_maps.append({"xt": xt_h, **shared})
    return in_maps, cs_all, scale, var


def _run(inputs, trace=False, trace_kwargs=None, mode=None):
    _ensure_concourse()
    from concourse.bass_utils import run_bass_kernel_spmd

    mode = mode or MODE
    in_maps, cs_all, scale, var = _host_prep(**inputs, mode=mode)
    nc = _get_nc(scale, mode)
    bkr = run_bass_kernel_spmd(
        nc,
        in_maps,
        list(range(NCORES)),
        trace=trace,
        **(trace_kwargs or {}),
    )
    mean = np.empty([N, P * G], np.float32)
    varr = np.empty([N, P * G], np.float32)
    for c in range(NCORES):
        o = np.asarray(bkr.results[c]["out"], np.float64)  # [2G, XL]
        cs = cs_all[c]  # [XL]
        m = o[:G] * cs  # [G, XL]
        v = var - o[G:] * (cs * cs)
        mean[:, c * PLOC * G : (c + 1) * PLOC * G] = (
            m.reshape(G, PLOC, N).transpose(2, 1, 0).reshape(N, PLOC * G)
        )
        varr[:, c * PLOC * G : (c + 1) * PLOC * G] = (
            v.reshape(G, PLOC, N).transpose(2, 1, 0).reshape(N, PLOC * G)
        )
    return mean, varr, bkr


def kernel(**inputs):
    mean, varr, _ = _run(inputs, trace=False)
    return mean, varr
